# revision 1
# baseline (speedup 1.0000x reference)
"""MLA (multi-head latent attention) Bass kernel for 8 TRN2 NeuronCores.

Sharding: 2-way data parallel over batch x 4-way tensor parallel over heads
(4 heads/core). Each core computes a partial output projection (transposed,
[DIM, S] fp16); the host sums the 4 head-group partials per batch in fp32 and
transposes.

v6 dataflow (per core): fp16 attention arithmetic; projections run on
fp8e4m3 DoubleRow matmuls. q_nope uses single-digit fp8 (softmax absorbs
its ~3% compute noise); the kv/rope projections and stage 3 use TWO-DIGIT
fp8 (hi = fp8(v), lo = fp8(v - hi) subnormal residual; three DR chains
hi.Whi + hi.Wlo + lo.Whi) giving ~0.1% error at 3/4 the fp16 column count.
Hardware-verified: the PE honors subnormal fp8 operands. The stage-3
operands (wo, OTP) are split host-side / fused into the normalization
multiply; OTP carries a x64 prescale (folded into the reciprocal row) so
late queries' shrunken outputs stay out of the fp8 subnormal range.

  stage 1 (two-tile software pipeline per 128-token tile):
    - q_nope = wq8.x8 (fp8 DR) -> fp16 QP sub0 (evac scale 1/WPRE)
    - [q_pe | k_pe] = x.wtok (two-digit fp8 DR, tables carry 1/WPRE),
      fused RoPE on DVE/Pool, PE-transposed -> QP sub1 / KP slot 4 (fp16)
    - kv latent = x.wkva (two-digit fp8 DR); rmsnorm (eps x WPRE^2)
      -> ckv16; PE-transpose -> ckvT fp16
    - K_abs[h] = wbk16[h].ckvT -> KP slots 0-3 (fp16)
    - V_absT[h] = ckvT.T.wbv16[h] -> VP (fp16)
  stage 2, flattened (cq, head, key tile) stream with 4-deep score
  lookahead so PE never stalls on exp:
    - scores.T tile = nope matmul (K=128) + rope matmul (K=64), fp16
    - additive -30000 causal mask on PSUM scores (diagonal tiles)
    - exp on ACT (scale=SCALE, bias -ln64 / -ln4 for cq=0 so early queries'
      denominators stay well away from underflow) -> fp16 pts
    - denominators: transposed 1-column matmuls (pts stationary) into a
      [128 query, 4] PSUM accumulator; reciprocal is PE-transposed back to
      a row and partition-broadcast
    - PV: fp16 matmuls against V_absT -> o tile [vdim, 512]
    - normalization fused into the OTP write
  stage 3 (one chunk behind, interleaved per head): out.T d-tile = 6
    two-digit fp8 DR matmuls against wo8h/wo8l x OTP8h/OTP8l; evac scales
    1/(64*64); per-d-tile DMA out.
"""

import numpy as np
import ml_dtypes

import concourse.bass as bass
import concourse.bacc as bacc
import concourse.mybir as mybir
import concourse.tile as tile
from concourse.bass_utils import run_bass_kernel_spmd

NPF8 = ml_dtypes.float8_e4m3
F8 = mybir.dt.float8e4
FP16 = mybir.dt.float16
FP32 = mybir.dt.float32
DRM = mybir.MatmulPerfMode.DoubleRow

B, S, DIM, H = 2, 2048, 2048, 16
KV_RANK, NOPE, ROPE, VDIM = 512, 128, 64, 128
QK = NOPE + ROPE
SCALE = QK ** -0.5
WPRE = 64.0             # fp8 weight prescale (avoids subnormals)
TP, DP = 4, 2
HL = H // TP            # heads per core = 4
P = 128
NT = S // P             # 16 token tiles
ND = DIM // P           # 16 dim subtiles
CH = 512                # token chunk
NCH = S // CH           # 4
NDP = DIM // 256        # 8 dim-subtile pairs
EPS = 1e-6
MASKNEG = -30000.0


def build_graph():
    nc = bacc.Bacc(None, target_bir_lowering=False)
    x8 = nc.declare_dram_parameter("x8", [P, NDP, 2, S], F8, isOutput=False)
    xlo = nc.declare_dram_parameter("xlo", [P, NDP, 2, S], F8,
                                    isOutput=False)
    wqP = nc.declare_dram_parameter("wqP", [P, NDP, 2, HL * NOPE], F8,
                                    isOutput=False)
    wtok8h = nc.declare_dram_parameter("wtok8h", [P, NDP, 2, 5 * ROPE], F8,
                                       isOutput=False)
    wtok8l = nc.declare_dram_parameter("wtok8l", [P, NDP, 2, 5 * ROPE], F8,
                                       isOutput=False)
    wkva8h = nc.declare_dram_parameter("wkva8h", [P, NDP, 2, KV_RANK], F8,
                                       isOutput=False)
    wkva8l = nc.declare_dram_parameter("wkva8l", [P, NDP, 2, KV_RANK], F8,
                                       isOutput=False)
    wbkH = nc.declare_dram_parameter("wbkH", [P, 4, HL, P], FP16,
                                     isOutput=False)
    wbvH = nc.declare_dram_parameter("wbvH", [P, 4, HL, P], FP16,
                                     isOutput=False)
    wo8h = nc.declare_dram_parameter("wo8h", [P, 4, DIM], F8,
                                     isOutput=False)
    wo8l = nc.declare_dram_parameter("wo8l", [P, 4, DIM], F8,
                                     isOutput=False)
    cosk5 = nc.declare_dram_parameter("cosk5", [P, 5, NT * 32], FP16,
                                      isOutput=False)
    sink5 = nc.declare_dram_parameter("sink5", [P, 5, NT * 32], FP16,
                                      isOutput=False)
    identH = nc.declare_dram_parameter("identH", [P, P], FP16, isOutput=False)
    maskA = nc.declare_dram_parameter("maskA", [P, P], FP16, isOutput=False)
    out = nc.declare_dram_parameter("out", [DIM, S], FP16, isOutput=True)

    with tile.TileContext(nc) as tc:
        with tc.tile_pool(name="persist", bufs=1) as pp:
            QP = pp.tile([P, 2, HL, S], FP16, tag="QP", name="QP")
            KP = pp.tile([P, HL + 1, S], FP16, tag="KP", name="KP")
            VP = pp.tile([P, NT, HL, VDIM], FP16, tag="VP", name="VP")
            OTP8h = pp.tile([P, 4, S], F8, tag="OTP8h", name="OTP8h")
            OTP8l = pp.tile([P, 4, S], F8, tag="OTP8l", name="OTP8l")
            identH_sb = pp.tile([P, P], FP16, tag="identH", name="identH_sb")
            maskA_sb = pp.tile([P, P], FP16, tag="maskA", name="maskA_sb")
            ones16 = pp.tile([P, 1], FP16, tag="ones16", name="ones16")
            eps_sb = pp.tile([P, 1], FP32, tag="eps", name="eps_sb")
            nb4 = pp.tile([P, 1], FP32, tag="nb4", name="nb4")
            nb64 = pp.tile([P, 1], FP32, tag="nb64", name="nb64")
            identF_sb = pp.tile([P, P], FP32, tag="identF",
                                name="identF_sb")
            wbk_sb = pp.tile([P, 4, HL, P], FP16, tag="wbk", name="wbk_sb")
            wbv_sb = pp.tile([P, 4, HL, P], FP16, tag="wbv", name="wbv_sb")

            nc.sync.dma_start(out=identH_sb[:], in_=identH[:])
            nc.sync.dma_start(out=maskA_sb[:], in_=maskA[:])
            nc.sync.dma_start(out=wbk_sb[:], in_=wbkH[:])
            nc.sync.dma_start(out=wbv_sb[:], in_=wbvH[:])
            nc.vector.memset(ones16[:], 1.0)
            nc.scalar.copy(identF_sb[:], identH_sb[:])
            nc.vector.memset(eps_sb[:], EPS * WPRE * WPRE)
            nc.vector.memset(nb4[:], -float(np.log(4.0)))
            nc.vector.memset(nb64[:], -float(np.log(64.0)))

            # ---------------- stage 1: projections ----------------
            with tc.tile_pool(name="s1", bufs=1) as s1, \
                 tc.tile_pool(name="s1b", bufs=2) as s1b, \
                 tc.tile_pool(name="s1x", bufs=1) as s1x, \
                 tc.tile_pool(name="s1qr", bufs=2, space="PSUM") as s1qr, \
                 tc.tile_pool(name="s1kv", bufs=2, space="PSUM") as s1kv, \
                 tc.tile_pool(name="s1qp", bufs=2, space="PSUM") as s1qp, \
                 tc.tile_pool(name="s1tq", bufs=1, space="PSUM") as s1tq, \
                 tc.tile_pool(name="s1tc", bufs=1, space="PSUM") as s1tc:
                cosk_sb = s1.tile([P, 5, NT * 32], FP16, tag="cosk",
                                  name="cosk_sb")
                sink_sb = s1.tile([P, 5, NT * 32], FP16, tag="sink",
                                  name="sink_sb")
                ckvTP = s1.tile([P, 4, S], FP16, tag="ckvTP", name="ckvTP")
                wq_sb = s1.tile([P, NDP, 2, HL * NOPE], F8, tag="wq",
                                name="wq_sb")
                wtokh_sb = s1.tile([P, NDP, 2, 5 * ROPE], F8, tag="wtokh",
                                   name="wtokh_sb")
                wtokl_sb = s1.tile([P, NDP, 2, 5 * ROPE], F8, tag="wtokl",
                                   name="wtokl_sb")
                wkvah_sb = s1.tile([P, NDP, 2, KV_RANK], F8, tag="wkvah",
                                   name="wkvah_sb")
                wkval_sb = s1.tile([P, NDP, 2, KV_RANK], F8, tag="wkval",
                                   name="wkval_sb")
                for dp in range(NDP):
                    nc.gpsimd.dma_start(out=wq_sb[:, dp, :, :],
                                        in_=wqP[:, dp, :, :])
                    nc.gpsimd.dma_start(out=wkvah_sb[:, dp, :, :],
                                        in_=wkva8h[:, dp, :, :])
                    nc.gpsimd.dma_start(out=wkval_sb[:, dp, :, :],
                                        in_=wkva8l[:, dp, :, :])
                nc.gpsimd.dma_start(out=wtokh_sb[:], in_=wtok8h[:])
                nc.gpsimd.dma_start(out=wtokl_sb[:], in_=wtok8l[:])
                nc.gpsimd.dma_start(out=cosk_sb[:], in_=cosk5[:])
                nc.gpsimd.dma_start(out=sink_sb[:], in_=sink5[:])
                def phase_mm(t, xc, xl):
                    """qpr + kv0 two-digit fp8 DR matmuls for tile t."""
                    tt = t % 4
                    qpr = s1qr.tile([P, 5, ROPE], FP32, tag="qpr",
                                    name="qpr")
                    chains = [(xc, wtokh_sb), (xc, wtokl_sb), (xl, wtokh_sb)]
                    for ci, (xs, ws) in enumerate(chains):
                        for dp in range(NDP):
                            nc.tensor.matmul(
                                qpr[:, :, :],
                                xs[:, dp, :, tt * P:(tt + 1) * P],
                                ws[:, dp, :, :],
                                start=(ci == 0 and dp == 0),
                                stop=(ci == 2 and dp == NDP - 1),
                                perf_mode=DRM)
                    kv0 = s1kv.tile([P, KV_RANK], FP32, tag="kv0",
                                    name="kv0")
                    chains = [(xc, wkvah_sb), (xc, wkval_sb), (xl, wkvah_sb)]
                    for ci, (xs, ws) in enumerate(chains):
                        for dp in range(NDP):
                            nc.tensor.matmul(
                                kv0[:],
                                xs[:, dp, :, tt * P:(tt + 1) * P],
                                ws[:, dp, :, :],
                                start=(ci == 0 and dp == 0),
                                stop=(ci == 2 and dp == NDP - 1),
                                perf_mode=DRM)
                    return qpr, kv0

                def phase_fin(t, qpr, kv0):
                    """rope, norm, transposes, K_abs/V_absT for tile t."""
                    csl = cosk_sb[:, :, t * 32:(t + 1) * 32]
                    ssl = sink_sb[:, :, t * 32:(t + 1) * 32]
                    xe = qpr[:, :, 0:32]
                    xo = qpr[:, :, 32:ROPE]
                    tm1 = s1b.tile([P, 5, 32], FP32, tag="tm1", name="tm1")
                    tm2 = s1b.tile([P, 5, 32], FP32, tag="tm2", name="tm2")
                    tm3 = s1b.tile([P, 5, 32], FP32, tag="tm3", name="tm3")
                    tm4 = s1b.tile([P, 5, 32], FP32, tag="tm4", name="tm4")
                    qpe_s = s1b.tile([P, 5, ROPE], FP16, tag="qpes",
                                     name="qpe_s")
                    nc.vector.tensor_mul(tm1[:], xe, csl)
                    nc.vector.tensor_mul(tm2[:], xo, ssl)
                    nc.vector.tensor_mul(tm4[:], xo, csl)
                    nc.vector.tensor_mul(tm3[:], xe, ssl)
                    nc.gpsimd.tensor_sub(qpe_s[:, :, 0:32], tm1[:], tm2[:])
                    nc.gpsimd.tensor_add(qpe_s[:, :, 32:ROPE], tm3[:],
                                         tm4[:])
                    # kv norm chain
                    sq = s1b.tile([P, KV_RANK], mybir.dt.bfloat16, tag="sq",
                                  name="sq")
                    red = s1b.tile([P, 1], FP32, tag="red", name="red")
                    nc.scalar.activation(
                        sq[:], kv0[:],
                        mybir.ActivationFunctionType.Square,
                        accum_out=red[:])
                    rms = s1b.tile([P, 1], FP32, tag="rms", name="rms")
                    nc.scalar.activation(rms[:], red[:],
                                         mybir.ActivationFunctionType.Sqrt,
                                         bias=eps_sb[:], scale=1.0 / KV_RANK)
                    rr = s1b.tile([P, 1], FP32, tag="rr", name="rr")
                    nc.vector.reciprocal(rr[:], rms[:])
                    ckv16 = s1b.tile([P, KV_RANK], FP16, tag="ckv16",
                                     name="ckv16")
                    nc.vector.tensor_scalar_mul(ckv16[:], kv0[:], rr[:])
                    # q_pe transposes (fp16, grouped PSUM bank)
                    t8q = s1tq.tile([ROPE, HL, P], FP16, tag="t8q",
                                    name="t8q")
                    for hh in range(HL):
                        nc.tensor.matmul(
                            t8q[:, hh, :], qpe_s[:, hh, :], identH_sb[:],
                            is_transpose=True,
                            start=(hh == 0), stop=(hh == HL - 1))
                    nc.vector.tensor_copy(
                        QP[0:ROPE, 1, :, t * P:(t + 1) * P], t8q[:])
                    # ckv + k_pe transposes (fp16, grouped PSUM bank)
                    t8c = s1tc.tile([P, 5, P], FP16, tag="t8c", name="t8c")
                    nc.tensor.matmul(
                        t8c[:, 0, :], ckv16[:, 0:P], identH_sb[:],
                        is_transpose=True, start=True, stop=False)
                    nc.tensor.matmul(
                        t8c[0:ROPE, 4, :], qpe_s[:, HL, :], identH_sb[:],
                        is_transpose=True, start=False, stop=False)
                    for l in range(1, 4):
                        nc.tensor.matmul(
                            t8c[:, l, :], ckv16[:, l * P:(l + 1) * P],
                            identH_sb[:], is_transpose=True,
                            start=False, stop=(l == 3))
                    nc.scalar.copy(ckvTP[:, :, t * P:(t + 1) * P],
                                   t8c[:, 0:4, :])
                    nc.vector.tensor_copy(
                        KP[0:ROPE, HL, t * P:(t + 1) * P], t8c[0:ROPE, 4, :])
                    # K_abs / V_absT
                    ka4 = s1qp.tile([P, HL, P], FP32, tag="qp", name="ka4")
                    for h in range(HL):
                        for l in range(4):
                            nc.tensor.matmul(
                                ka4[:, h, :],
                                wbk_sb[:, l, h, :],
                                ckvTP[:, l, t * P:(t + 1) * P],
                                start=(h == 0 and l == 0),
                                stop=(h == HL - 1 and l == 3))
                    nc.scalar.copy(KP[:, 0:HL, t * P:(t + 1) * P],
                                   ka4[:])
                    vt4 = s1qp.tile([P, HL, P], FP32, tag="qp", name="vt4")
                    for h in range(HL):
                        for l in range(4):
                            nc.tensor.matmul(
                                vt4[:, h, :],
                                ckvTP[:, l, t * P:(t + 1) * P],
                                wbv_sb[:, l, h, :],
                                start=(h == 0 and l == 0),
                                stop=(h == HL - 1 and l == 3))
                    nc.vector.tensor_copy(VP[:, t, :, :], vt4[:])

                prev = None
                xc = xh = None
                for t in range(NT):
                    if t % 4 == 0:
                        c1 = t // 4
                        xc = s1b.tile([P, NDP, 2, CH], F8, tag="xc",
                                      name="xc")
                        xl = s1b.tile([P, NDP, 2, CH], F8, tag="xl",
                                      name="xl")
                        for dp in range(NDP):
                            qe = nc.sync if dp % 2 == 0 else nc.scalar
                            qe.dma_start(
                                out=xc[:, dp, :, :],
                                in_=x8[:, dp, :, c1 * CH:(c1 + 1) * CH])
                        for dp in range(NDP):
                            qe2 = nc.scalar if dp % 2 == 0 else nc.sync
                            qe2.dma_start(
                                out=xl[:, dp, :, :],
                                in_=xlo[:, dp, :, c1 * CH:(c1 + 1) * CH])
                        for ft in range(HL):
                            qp = s1qp.tile([P, CH], FP32, tag="qp",
                                           name="qp")
                            for dp in range(NDP):
                                nc.tensor.matmul(
                                    qp[:],
                                    wq_sb[:, dp, :, ft * P:(ft + 1) * P],
                                    xc[:, dp, :, :],
                                    start=(dp == 0), stop=(dp == NDP - 1),
                                    perf_mode=DRM)
                            nc.scalar.activation(
                                QP[:, 0, ft, c1 * CH:(c1 + 1) * CH], qp[:],
                                mybir.ActivationFunctionType.Copy,
                                scale=1.0 / WPRE)
                    cur = phase_mm(t, xc, xl)
                    if prev is not None:
                        phase_fin(t - 1, *prev)
                    prev = cur
                phase_fin(NT - 1, *prev)

            # ---------------- stage 2: attention -------------------
            with tc.tile_pool(name="s2", bufs=1) as s2, \
                 tc.tile_pool(name="s2b", bufs=3) as s2b, \
                 tc.tile_pool(name="ps_sp", bufs=4, space="PSUM") as ps_sp, \
                 tc.tile_pool(name="ps_op", bufs=3, space="PSUM") as ps_op, \
                 tc.tile_pool(name="ps_sum", bufs=1, space="PSUM") as ps_sum:
                woh_sb = s2.tile([P, 4, DIM], F8, tag="woh", name="woh_sb")
                wol_sb = s2.tile([P, 4, DIM], F8, tag="wol", name="wol_sb")
                nc.sync.dma_start(out=woh_sb[:], in_=wo8h[:])
                nc.sync.dma_start(out=wol_sb[:], in_=wo8l[:])
                def emit_scores(cq, h, ti, bias_t):
                    off = max(ti - 4 * cq, 0) * P
                    sp = ps_sp.tile([P, CH], FP32, tag="sp", name="sp")
                    kc = slice(ti * P, (ti + 1) * P)
                    nc.tensor.matmul(
                        sp[:, off:CH],
                        KP[:, h, kc],
                        QP[:, 0, h, cq * CH + off:(cq + 1) * CH],
                        start=True, stop=False)
                    nc.tensor.matmul(
                        sp[:, off:CH],
                        KP[0:ROPE, HL, kc],
                        QP[0:ROPE, 1, h, cq * CH + off:(cq + 1) * CH],
                        start=False, stop=True)
                    if ti - 4 * cq >= 0:
                        nc.vector.tensor_add(sp[:, off:off + P],
                                             sp[:, off:off + P], maskA_sb[:])
                    pts = s2b.tile([P, CH], FP16, tag="pts",
                                   name="pts", bufs=6)
                    nc.scalar.activation(
                        pts[:, off:CH], sp[:, off:CH],
                        mybir.ActivationFunctionType.Exp,
                        bias=bias_t[:], scale=SCALE)
                    return pts, off

                def emit_stage3(cq, d):
                    outp = ps_op.tile([P, CH], FP32, tag="op", name="outp")
                    cols = slice((cq - 1) * CH, cq * CH)
                    chains = [(woh_sb, OTP8h), (woh_sb, OTP8l),
                              (wol_sb, OTP8h)]
                    for ci, (wsb, osb) in enumerate(chains):
                        for j in range(2):
                            nc.tensor.matmul(
                                outp[:],
                                wsb[:, 2 * j:2 * j + 2, d * P:(d + 1) * P],
                                osb[:, 2 * j:2 * j + 2, cols],
                                start=(ci == 0 and j == 0),
                                stop=(ci == 2 and j == 1),
                                perf_mode=DRM)
                    oc = s2b.tile([P, CH], FP16, tag="oc", name="oc",
                                  bufs=3)
                    if d % 2 == 0:
                        nc.vector.tensor_scalar_mul(oc[:], outp[:],
                                                    1.0 / (WPRE * WPRE))
                    else:
                        nc.scalar.activation(
                            oc[:], outp[:],
                            mybir.ActivationFunctionType.Copy,
                            scale=1.0 / (WPRE * WPRE))
                    nc.sync.dma_start(
                        out=out[d * P:(d + 1) * P, cols],
                        in_=oc[:])

                items = [(cq, h, ti) for cq in range(NCH)
                         for h in range(HL) for ti in range(4 * cq + 4)]
                state = {}
                pend = []
                for idx, (cq, h, ti) in enumerate(items):
                    ntk = 4 * cq + 4
                    if ti == 0:
                        state[(cq, h)] = (
                            ps_op.tile([P, CH], FP32, tag="op",
                                       name="op_ps"),
                            ps_sum.tile([P, 4], FP32, tag="sums",
                                        name="sums_t"))
                    op_ps, sums_t = state[(cq, h)]
                    while len(pend) < 4 and idx + len(pend) < len(items):
                        cq2, h2, t2 = items[idx + len(pend)]
                        pend.append(emit_scores(cq2, h2, t2,
                                                nb4 if cq2 == 0 else nb64))
                    pts, off = pend.pop(0)
                    if cq > 0 and ti < 4:
                        emit_stage3(cq, 4 * h + ti)
                    for cc in range(off // P, 4):
                        nc.tensor.matmul(
                            sums_t[:, cc:cc + 1],
                            pts[:, cc * P:(cc + 1) * P],
                            ones16[:],
                            start=(ti == 0 and cc == 0),
                            stop=(ti == ntk - 1 and cc == 3))
                    nc.tensor.matmul(
                        op_ps[:, off:CH],
                        VP[:, ti, h, :],
                        pts[:, off:CH],
                        start=(ti == 0), stop=(ti == ntk - 1))
                    if ti == ntk - 1:
                        # normalization fused into OTP write
                        sums_f = s2b.tile([P, 4], FP32, tag="sums_f",
                                          name="sums_f")
                        nc.vector.tensor_scalar_add(sums_f[:], sums_t[:],
                                                    1e-20)
                        rec_tf = s2b.tile([P, 4], FP32, tag="rec_tf",
                                          name="rec_tf")
                        nc.vector.reciprocal(rec_tf[:], sums_f[:])
                        rec_tp = ps_op.tile([1, 4, P], FP32, tag="op",
                                            name="rec_tp")
                        for cc in range(4):
                            nc.tensor.matmul(rec_tp[:, cc, :],
                                             rec_tf[:, cc:cc + 1],
                                             identF_sb[:],
                                             is_transpose=True,
                                             start=(cc == 0), stop=(cc == 3))
                        rec_r = s2b.tile([1, 4, P], FP16, tag="rec_r",
                                         name="rec_r")
                        nc.scalar.activation(
                            rec_r[:], rec_tp[:],
                            mybir.ActivationFunctionType.Copy, scale=WPRE)
                        recip_bc = s2b.tile([P, CH], FP16, tag="recip_bc",
                                            name="recip_bc")
                        nc.gpsimd.partition_broadcast(recip_bc[:],
                                                      rec_r[0:1, :, :])
                        ot16 = s2b.tile([P, CH], FP16, tag="ot16",
                                        name="ot16", bufs=2)
                        nc.vector.tensor_mul(ot16[:], op_ps[:], recip_bc[:])
                        occ = cq * CH
                        nc.gpsimd.tensor_copy(
                            OTP8h[:, h, occ:occ + CH], ot16[:])
                        nc.vector.scalar_tensor_tensor(
                            OTP8l[:, h, occ:occ + CH], ot16[:], 1.0,
                            OTP8h[:, h, occ:occ + CH],
                            op0=mybir.AluOpType.mult,
                            op1=mybir.AluOpType.subtract)
                        del state[(cq, h)]
                # -------- stage 3 for the final chunk --------
                for d in range(ND):
                    emit_stage3(NCH, d)
    nc.finalize()
    return nc


_NC = None


def _get_nc():
    global _NC
    if _NC is None:
        _NC = build_graph()
    return _NC


def _pair_dim(a):
    """[X*256, N] -> [128, X, 2, N]: pair 128-row subtiles for DoubleRow."""
    x2, n = a.shape
    return np.ascontiguousarray(
        a.reshape(x2 // 256, 2, 128, n).transpose(2, 0, 1, 3))


def _sub_dim(a):
    """[X*128, N] -> [128, X, N]: 128-row subtiles, partition-major."""
    x1, n = a.shape
    return np.ascontiguousarray(a.reshape(x1 // 128, 128, n).transpose(1, 0, 2))


def _prep_core_inputs(x, wq, wkv_a, kv_norm_w, wkv_b, wo, cos, sin):
    """Host-side shard prep. Returns list of 8 in_maps (core = b*4 + g)."""
    perm = np.concatenate([np.arange(0, ROPE, 2), np.arange(1, ROPE, 2)])
    cosf = np.asarray(cos, np.float32)
    sinf = np.asarray(sin, np.float32)
    cosk = np.ascontiguousarray(
        cosf.reshape(NT, P, ROPE // 2).transpose(1, 0, 2).reshape(P, -1))
    sink = np.ascontiguousarray(
        sinf.reshape(NT, P, ROPE // 2).transpose(1, 0, 2).reshape(P, -1))
    cosk5 = np.ascontiguousarray(np.broadcast_to(
        cosk[:, None, :] / WPRE, (P, 5, NT * 32))).astype(np.float16)
    sink5 = np.ascontiguousarray(np.broadcast_to(
        sink[:, None, :] / WPRE, (P, 5, NT * 32))).astype(np.float16)
    identH = np.eye(P, dtype=np.float16)
    k_idx = np.arange(P)[:, None]
    q_idx = np.arange(P)[None, :]
    maskA = np.where(q_idx >= k_idx, 0.0, MASKNEG).astype(np.float16)

    w = np.asarray(kv_norm_w, np.float32)
    wq_np = np.asarray(wq, np.float32)
    wkva_np = np.asarray(wkv_a, np.float32)
    wkvb_np = np.asarray(wkv_b, np.float32)
    wo_np = np.asarray(wo, np.float32)
    x_np = np.asarray(x, np.float32)

    def _hilo(rows):
        """[R, DIM] -> paired fp8 hi + residual lo (x WPRE)."""
        wT = np.ascontiguousarray(rows.T) * WPRE
        hi = wT.astype(NPF8)
        lo = (wT - hi.astype(np.float32)).astype(NPF8)
        return _pair_dim(hi.astype(np.float32)).astype(NPF8), \
               _pair_dim(lo.astype(np.float32)).astype(NPF8)

    wkva8h, wkva8l = _hilo(wkva_np[:KV_RANK])
    kpe_rows = wkva_np[KV_RANK:][perm]                    # [64, DIM]

    x8s, xlos = [], []
    for b in range(B):
        xT = np.ascontiguousarray(x_np[b].T)
        hi = xT.astype(NPF8)
        lo = (xT - hi.astype(np.float32)).astype(NPF8)
        x8s.append(_pair_dim(hi.astype(np.float32)).astype(NPF8))
        xlos.append(_pair_dim(lo.astype(np.float32)).astype(NPF8))

    wq_h = wq_np.reshape(H, QK, DIM)
    wb = wkvb_np.reshape(H, NOPE + VDIM, KV_RANK)

    in_maps = []
    _gcache = {}
    for c in range(DP * TP):
        b, g = c // TP, c % TP
        if g in _gcache:
            m = dict(_gcache[g])
            m["x8"] = x8s[b]
            m["xlo"] = xlos[b]
            in_maps.append(m)
            continue
        hs = list(range(g * HL, (g + 1) * HL))
        nope_rows = wq_h[hs, :NOPE].reshape(HL * NOPE, DIM) * WPRE
        wqPa = _pair_dim(np.ascontiguousarray(nope_rows.T)).astype(NPF8)
        rope_rows = wq_h[hs, NOPE:][:, perm].reshape(HL * ROPE, DIM)
        wtok = np.concatenate([rope_rows, kpe_rows], axis=0)   # [320, DIM]
        wtok8h, wtok8l = _hilo(wtok)
        # wbkH[p, l, h, n] = (wb_k[h] w)[n, l*128+p]
        wbk = np.stack([wb[hh, :NOPE] * w[None, :] for hh in hs])
        wbkH = np.ascontiguousarray(
            wbk.transpose(2, 0, 1).reshape(4, P, HL, P)
            .transpose(1, 0, 2, 3)).astype(np.float16)
        wbv = np.stack([wb[hh, NOPE:] * w[None, :] for hh in hs])
        wbvH = np.ascontiguousarray(
            wbv.transpose(2, 0, 1).reshape(4, P, HL, P)
            .transpose(1, 0, 2, 3)).astype(np.float16)
        wo_g = wo_np[:, g * HL * VDIM:(g + 1) * HL * VDIM]
        woT = np.ascontiguousarray(
            wo_g.T.reshape(4, P, DIM).transpose(1, 0, 2)) * WPRE
        wo8ha = woT.astype(NPF8)
        wo8la = (woT - wo8ha.astype(np.float32)).astype(NPF8)
        _gcache[g] = {
            "x8": x8s[b], "xlo": xlos[b], "wqP": wqPa,
            "wtok8h": wtok8h, "wtok8l": wtok8l,
            "wkva8h": wkva8h, "wkva8l": wkva8l,
            "wbkH": wbkH, "wbvH": wbvH, "wo8h": wo8ha, "wo8l": wo8la,
            "cosk5": cosk5, "sink5": sink5,
            "identH": identH, "maskA": maskA,
        }
        in_maps.append(_gcache[g])
    return in_maps


def run(inputs, trace=False, **kw):
    nc = _get_nc()
    in_maps = _prep_core_inputs(**inputs)
    res = run_bass_kernel_spmd(nc, in_maps, list(range(DP * TP)),
                               trace=trace, **kw)
    outs = [r["out"] for r in res.results]
    full = np.empty((B, S, DIM), np.float32)
    for b in range(B):
        acc = outs[b * TP].astype(np.float32)
        for g in range(1, TP):
            acc = acc + outs[b * TP + g].astype(np.float32)
        full[b] = acc.T
    return full, res


def kernel(**inputs):
    inputs = {k: np.asarray(v) for k, v in inputs.items()}
    full, _ = run(inputs)
    return full



# revision 30
# speedup vs baseline: 1.0325x; 1.0325x over previous
"""MLA (multi-head latent attention) Bass kernel for 8 TRN2 NeuronCores.

Sharding: 2-way data parallel over batch x 4-way tensor parallel over heads
(4 heads/core). Each core computes a partial output projection (transposed,
[DIM, S] fp16); the host sums the 4 head-group partials per batch in fp32 and
transposes.

v6 dataflow (per core): fp16 attention arithmetic; projections run on
fp8e4m3 DoubleRow matmuls. q_nope uses single-digit fp8 (softmax absorbs
its ~3% compute noise); the kv/rope projections and stage 3 use TWO-DIGIT
fp8 (hi = fp8(v), lo = fp8(v - hi) subnormal residual; three DR chains
hi.Whi + hi.Wlo + lo.Whi) giving ~0.1% error at 3/4 the fp16 column count.
Hardware-verified: the PE honors subnormal fp8 operands. The stage-3
operands (wo, OTP) are split host-side / fused into the normalization
multiply; OTP carries a x64 prescale (folded into the reciprocal row) so
late queries' shrunken outputs stay out of the fp8 subnormal range.

  stage 1 (two-tile software pipeline per 128-token tile):
    - q_nope = wq8.x8 (fp8 DR) -> fp16 QP sub0 (evac scale 1/WPRE)
    - [q_pe | k_pe] = x.wtok (two-digit fp8 DR, tables carry 1/WPRE),
      fused RoPE on DVE/Pool, PE-transposed -> QP sub1 / KP slot 4 (fp16)
    - kv latent = x.wkva (two-digit fp8 DR); rmsnorm (eps x WPRE^2)
      -> ckv16; PE-transpose -> ckvT fp16
    - K_abs[h] = wbk16[h].ckvT -> KP slots 0-3 (fp16)
    - V_absT[h] = ckvT.T.wbv16[h] -> VP (fp16)
  stage 2, flattened (cq, head, key tile) stream with 4-deep score
  lookahead so PE never stalls on exp:
    - scores.T tile = nope matmul (K=128) + rope matmul (K=64), fp16
    - additive -30000 causal mask on PSUM scores (diagonal tiles)
    - exp on ACT (scale=SCALE, bias -ln64 / -ln4 for cq=0 so early queries'
      denominators stay well away from underflow) -> fp16 pts
    - denominators: transposed 1-column matmuls (pts stationary) into a
      [128 query, 4] PSUM accumulator; reciprocal is PE-transposed back to
      a row and partition-broadcast
    - PV: fp16 matmuls against V_absT -> o tile [vdim, 512]
    - normalization fused into the OTP write
  stage 3 (one chunk behind, interleaved per head): out.T d-tile = 6
    two-digit fp8 DR matmuls against wo8h/wo8l x OTP8h/OTP8l; evac scales
    1/(64*64); per-d-tile DMA out.
"""

import numpy as np
import ml_dtypes

import concourse.bass as bass
import concourse.bacc as bacc
import concourse.mybir as mybir
import concourse.tile as tile
from concourse.bass_utils import run_bass_kernel_spmd

NPF8 = ml_dtypes.float8_e4m3
F8 = mybir.dt.float8e4
FP16 = mybir.dt.float16
FP32 = mybir.dt.float32
DRM = mybir.MatmulPerfMode.DoubleRow

B, S, DIM, H = 2, 2048, 2048, 16
KV_RANK, NOPE, ROPE, VDIM = 512, 128, 64, 128
QK = NOPE + ROPE
SCALE = QK ** -0.5
WPRE = 64.0             # fp8 weight prescale (avoids subnormals)
TP, DP = 4, 2
HL = H // TP            # heads per core = 4
P = 128
NT = S // P             # 16 token tiles
ND = DIM // P           # 16 dim subtiles
CH = 512                # token chunk
NCH = S // CH           # 4
NDP = DIM // 256        # 8 dim-subtile pairs
EPS = 1e-6
MASKNEG = -30000.0
# fp8 prescales for the two-digit DR attention operands
CPRE = 16.0             # ckvT8 latent
WBPRE = 1024.0          # wkv_b halves (host-side)
QKPRE = 32.0            # QP8 / KP8 score operands (products carry 1024)
SPROD = QKPRE * QKPRE   # score PSUM carries true*1024
KAPSC = QKPRE / (CPRE * WBPRE)   # K_abs PSUM -> KP8 hi evac scale (1/512)
VPSC = 1.0 / (CPRE * WBPRE)      # V_abs PSUM -> VP fp16 evac scale


def build_graph():
    nc = bacc.Bacc(None, target_bir_lowering=False)
    x8 = nc.declare_dram_parameter("x8", [P, NDP, 2, S], F8, isOutput=False)
    xlo = nc.declare_dram_parameter("xlo", [P, NDP, 2, S], F8,
                                    isOutput=False)
    wqP = nc.declare_dram_parameter("wqP", [P, NDP, 2, HL * NOPE], F8,
                                    isOutput=False)
    wtok8h = nc.declare_dram_parameter("wtok8h", [P, NDP, 2, 5 * ROPE], F8,
                                       isOutput=False)
    wtok8l = nc.declare_dram_parameter("wtok8l", [P, NDP, 2, 5 * ROPE], F8,
                                       isOutput=False)
    wkva8h = nc.declare_dram_parameter("wkva8h", [P, NDP, 2, KV_RANK], F8,
                                       isOutput=False)
    wkva8l = nc.declare_dram_parameter("wkva8l", [P, NDP, 2, KV_RANK], F8,
                                       isOutput=False)
    wbk8h = nc.declare_dram_parameter("wbk8h", [P, 2, 2, HL, P], F8,
                                      isOutput=False)
    wbk8l = nc.declare_dram_parameter("wbk8l", [P, 2, 2, HL, P], F8,
                                      isOutput=False)
    wbv8h = nc.declare_dram_parameter("wbv8h", [P, 2, 2, HL, P], F8,
                                      isOutput=False)
    wbv8l = nc.declare_dram_parameter("wbv8l", [P, 2, 2, HL, P], F8,
                                      isOutput=False)
    wo8h = nc.declare_dram_parameter("wo8h", [P, 4, DIM], F8,
                                     isOutput=False)
    wo8l = nc.declare_dram_parameter("wo8l", [P, 4, DIM], F8,
                                     isOutput=False)
    cosk5 = nc.declare_dram_parameter("cosk5", [P, 5, NT * 32], FP16,
                                      isOutput=False)
    sink5 = nc.declare_dram_parameter("sink5", [P, 5, NT * 32], FP16,
                                      isOutput=False)
    identH = nc.declare_dram_parameter("identH", [P, P], FP16, isOutput=False)
    maskA = nc.declare_dram_parameter("maskA", [P, P], FP32, isOutput=False)
    out = nc.declare_dram_parameter("out", [DIM, S], FP16, isOutput=True)

    with tile.TileContext(nc) as tc:
        with tc.tile_pool(name="persist", bufs=1) as pp:
            # QP8 slots: {0: qn hi, 1: qp hi, 2: qn lo, 3: qp lo} (x QKPRE)
            QP8 = pp.tile([P, 4, HL, S], F8, tag="QP8", name="QP8")
            # KP8 x-slots: 0..3 K_abs[h] hi, 4 k_pe hi, 5..8 K_abs[h] lo,
            # 9 k_pe lo.  k_pe rows live on partitions 0:64; 64:128 zeroed.
            KP8 = pp.tile([P, 2 * HL + 2, S], F8, tag="KP8", name="KP8")
            VP = pp.tile([P, NT, HL, VDIM], FP16, tag="VP", name="VP")
            OTP8h = pp.tile([P, 4, S], F8, tag="OTP8h", name="OTP8h")
            OTP8l = pp.tile([P, 4, S], F8, tag="OTP8l", name="OTP8l")
            identH_sb = pp.tile([P, P], FP16, tag="identH", name="identH_sb")
            maskA_sb = pp.tile([P, P], FP32, tag="maskA", name="maskA_sb")
            ones16 = pp.tile([P, 1], FP16, tag="ones16", name="ones16")
            eps_sb = pp.tile([P, 1], FP32, tag="eps", name="eps_sb")
            nb4 = pp.tile([P, 1], FP32, tag="nb4", name="nb4")
            nb64 = pp.tile([P, 1], FP32, tag="nb64", name="nb64")
            identF_sb = pp.tile([P, P], FP32, tag="identF",
                                name="identF_sb")
            wbk8h_sb = pp.tile([P, 2, 2, HL, P], F8, tag="wbk8h",
                               name="wbk8h_sb")
            wbk8l_sb = pp.tile([P, 2, 2, HL, P], F8, tag="wbk8l",
                               name="wbk8l_sb")
            wbv8h_sb = pp.tile([P, 2, 2, HL, P], F8, tag="wbv8h",
                               name="wbv8h_sb")
            wbv8l_sb = pp.tile([P, 2, 2, HL, P], F8, tag="wbv8l",
                               name="wbv8l_sb")
            # two-digit fp8 latent ckvT (x CPRE), DR slot layout:
            # [p, lat-pair, slot, tok] with lat block = 2*lp + slot
            ckvT8h = pp.tile([P, 2, 2, S], F8, tag="ckvT8h",
                             name="ckvT8h")
            ckvT8l = pp.tile([P, 2, 2, S], F8, tag="ckvT8l",
                             name="ckvT8l")

            nc.vector.memset(ones16[:], 1.0)
            nc.scalar.copy(identF_sb[:], identH_sb[:])
            nc.vector.memset(eps_sb[:], EPS * WPRE * WPRE)
            nc.vector.memset(nb4[:], -float(np.log(4.0)))
            nc.vector.memset(nb64[:], -float(np.log(64.0)))
            # zero the dead rope rows of the shared k_pe slots so the
            # score DR passes can read QP8 garbage there harmlessly
            nc.vector.memset(KP8[ROPE:P, HL, :], 0.0)
            nc.vector.memset(KP8[ROPE:P, 2 * HL + 1, :], 0.0)
            nc.vector.memset(QP8[ROPE:P, 1, :, :], 0.0)
            nc.vector.memset(QP8[ROPE:P, 3, :, :], 0.0)

            def emit_kabs(t, pool, tag):
                """K_abs for token tile t via two-digit fp8 DR over the
                latent; evac into KP8 (two-digit)."""
                ts = slice(t * P, (t + 1) * P)
                ka4 = pool.tile([P, HL, P], FP32, tag=tag, name="ka4")
                kchains = [(wbk8h_sb, ckvT8h), (wbk8l_sb, ckvT8h),
                           (wbk8h_sb, ckvT8l)]
                for h in range(HL):
                    for ci, (wsb, csb) in enumerate(kchains):
                        for lp in range(2):
                            nc.tensor.matmul(
                                ka4[:, h, :],
                                wsb[:, lp, :, h, :],
                                csb[:, lp, :, ts],
                                start=(h == 0 and ci == 0 and lp == 0),
                                stop=(h == HL - 1 and ci == 2 and lp == 1),
                                perf_mode=DRM)
                for h in range(HL):
                    nc.scalar.activation(
                        KP8[:, h, ts], ka4[:, h, :],
                        mybir.ActivationFunctionType.Copy, scale=KAPSC)
                    nc.vector.scalar_tensor_tensor(
                        KP8[:, HL + 1 + h, ts], ka4[:, h, :], KAPSC,
                        KP8[:, h, ts],
                        op0=mybir.AluOpType.mult,
                        op1=mybir.AluOpType.subtract)

            def emit_vabs(t, pool, tag):
                """V_absT for token tile t -> VP (fp16)."""
                ts = slice(t * P, (t + 1) * P)
                vt4 = pool.tile([P, HL, P], FP32, tag=tag, name="vt4")
                vchains = [(ckvT8h, wbv8h_sb), (ckvT8h, wbv8l_sb),
                           (ckvT8l, wbv8h_sb)]
                for h in range(HL):
                    for ci, (csb, wsb) in enumerate(vchains):
                        for lp in range(2):
                            nc.tensor.matmul(
                                vt4[:, h, :],
                                csb[:, lp, :, ts],
                                wsb[:, lp, :, h, :],
                                start=(h == 0 and ci == 0 and lp == 0),
                                stop=(h == HL - 1 and ci == 2 and lp == 1),
                                perf_mode=DRM)
                nc.vector.tensor_scalar_mul(VP[:, t, :, :], vt4[:], VPSC)

            # ---------------- stage 1: projections ----------------
            with tc.tile_pool(name="s1", bufs=1) as s1, \
                 tc.tile_pool(name="s1b", bufs=2) as s1b, \
                 tc.tile_pool(name="s1x", bufs=1) as s1x, \
                 tc.tile_pool(name="s1qr", bufs=2, space="PSUM") as s1qr, \
                 tc.tile_pool(name="s1kv", bufs=2, space="PSUM") as s1kv, \
                 tc.tile_pool(name="s1qp", bufs=2, space="PSUM") as s1qp, \
                 tc.tile_pool(name="s1tq", bufs=1, space="PSUM") as s1tq, \
                 tc.tile_pool(name="s1tc", bufs=1, space="PSUM") as s1tc:
                cosk_sb = s1.tile([P, 5, NT * 32], FP16, tag="cosk",
                                  name="cosk_sb")
                sink_sb = s1.tile([P, 5, NT * 32], FP16, tag="sink",
                                  name="sink_sb")
                wq_sb = s1.tile([P, NDP, 2, HL * NOPE], F8, tag="wq",
                                name="wq_sb")
                wtokh_sb = s1.tile([P, NDP, 2, 5 * ROPE], F8, tag="wtokh",
                                   name="wtokh_sb")
                wtokl_sb = s1.tile([P, NDP, 2, 5 * ROPE], F8, tag="wtokl",
                                   name="wtokl_sb")
                wkvah_sb = s1.tile([P, NDP, 2, KV_RANK], F8, tag="wkvah",
                                   name="wkvah_sb")
                wkval_sb = s1.tile([P, NDP, 2, KV_RANK], F8, tag="wkval",
                                   name="wkval_sb")
                nc.gpsimd.dma_start(out=identH_sb[:], in_=identH[:])
                for dp in range(NDP):
                    nc.gpsimd.dma_start(out=wq_sb[:, dp, :, :],
                                        in_=wqP[:, dp, :, :])
                nc.gpsimd.dma_start(out=wtokh_sb[:], in_=wtok8h[:])
                nc.gpsimd.dma_start(out=wtokl_sb[:], in_=wtok8l[:])
                for dp in range(NDP):
                    nc.gpsimd.dma_start(out=wkvah_sb[:, dp, :, :],
                                        in_=wkva8h[:, dp, :, :])
                    nc.gpsimd.dma_start(out=wkval_sb[:, dp, :, :],
                                        in_=wkva8l[:, dp, :, :])
                nc.gpsimd.dma_start(out=cosk_sb[:], in_=cosk5[:])
                nc.gpsimd.dma_start(out=sink_sb[:], in_=sink5[:])
                def phase_mm(t, xc, xl):
                    """qpr + kv0 two-digit fp8 DR matmuls for tile t."""
                    tt = t % 4
                    qpr = s1qr.tile([P, 5, ROPE], FP32, tag="qpr",
                                    name="qpr")
                    chains = [(xc, wtokh_sb), (xc, wtokl_sb), (xl, wtokh_sb)]
                    for ci, (xs, ws) in enumerate(chains):
                        for dp in range(NDP):
                            nc.tensor.matmul(
                                qpr[:, :, :],
                                xs[:, dp, :, tt * P:(tt + 1) * P],
                                ws[:, dp, :, :],
                                start=(ci == 0 and dp == 0),
                                stop=(ci == 2 and dp == NDP - 1),
                                perf_mode=DRM)
                    kv0 = s1kv.tile([P, KV_RANK], FP32, tag="kv0",
                                    name="kv0")
                    chains = [(xc, wkvah_sb), (xc, wkval_sb), (xl, wkvah_sb)]
                    for ci, (xs, ws) in enumerate(chains):
                        for dp in range(NDP):
                            nc.tensor.matmul(
                                kv0[:],
                                xs[:, dp, :, tt * P:(tt + 1) * P],
                                ws[:, dp, :, :],
                                start=(ci == 0 and dp == 0),
                                stop=(ci == 2 and dp == NDP - 1),
                                perf_mode=DRM)
                    return qpr, kv0

                def phase_fin(t, qpr, kv0):
                    """rope, norm, transposes, K_abs/V_absT for tile t."""
                    csl = cosk_sb[:, :, t * 32:(t + 1) * 32]
                    ssl = sink_sb[:, :, t * 32:(t + 1) * 32]
                    xe = qpr[:, :, 0:32]
                    xo = qpr[:, :, 32:ROPE]
                    tm1 = s1b.tile([P, 5, 32], FP32, tag="tm1", name="tm1")
                    tm2 = s1b.tile([P, 5, 32], FP32, tag="tm2", name="tm2")
                    tm3 = s1b.tile([P, 5, 32], FP32, tag="tm3", name="tm3")
                    tm4 = s1b.tile([P, 5, 32], FP32, tag="tm4", name="tm4")
                    qpe_s = s1b.tile([P, 5, ROPE], FP16, tag="qpes",
                                     name="qpe_s")
                    nc.vector.tensor_mul(tm1[:], xe, csl)
                    nc.vector.tensor_mul(tm2[:], xo, ssl)
                    nc.vector.tensor_mul(tm4[:], xo, csl)
                    nc.vector.tensor_mul(tm3[:], xe, ssl)
                    nc.gpsimd.tensor_sub(qpe_s[:, :, 0:32], tm1[:], tm2[:])
                    nc.gpsimd.tensor_add(qpe_s[:, :, 32:ROPE], tm3[:],
                                         tm4[:])
                    # kv norm chain
                    sq = s1b.tile([P, KV_RANK], mybir.dt.bfloat16, tag="sq",
                                  name="sq")
                    red = s1b.tile([P, 1], FP32, tag="red", name="red")
                    nc.scalar.activation(
                        sq[:], kv0[:],
                        mybir.ActivationFunctionType.Square,
                        accum_out=red[:])
                    rms = s1b.tile([P, 1], FP32, tag="rms", name="rms")
                    nc.scalar.activation(rms[:], red[:],
                                         mybir.ActivationFunctionType.Sqrt,
                                         bias=eps_sb[:], scale=1.0 / KV_RANK)
                    rr = s1b.tile([P, 1], FP32, tag="rr", name="rr")
                    nc.vector.reciprocal(rr[:], rms[:])
                    ckv16 = s1b.tile([P, KV_RANK], FP16, tag="ckv16",
                                     name="ckv16")
                    nc.vector.tensor_scalar_mul(ckv16[:], kv0[:], rr[:])
                    # q_pe transposes (fp16, grouped PSUM bank)
                    t8q = s1tq.tile([ROPE, HL, P], FP16, tag="t8q",
                                    name="t8q")
                    for hh in range(HL):
                        nc.tensor.matmul(
                            t8q[:, hh, :], qpe_s[:, hh, :], identH_sb[:],
                            is_transpose=True,
                            start=(hh == 0), stop=(hh == HL - 1))
                    ts = slice(t * P, (t + 1) * P)
                    nc.vector.tensor_scalar_mul(
                        QP8[0:ROPE, 1, :, ts], t8q[:], QKPRE)
                    nc.vector.scalar_tensor_tensor(
                        QP8[0:ROPE, 3, :, ts], t8q[:], QKPRE,
                        QP8[0:ROPE, 1, :, ts],
                        op0=mybir.AluOpType.mult,
                        op1=mybir.AluOpType.subtract)
                    # ckv + k_pe transposes (fp16, grouped PSUM bank)
                    t8c = s1tc.tile([P, 5, P], FP16, tag="t8c", name="t8c")
                    nc.tensor.matmul(
                        t8c[:, 0, :], ckv16[:, 0:P], identH_sb[:],
                        is_transpose=True, start=True, stop=False)
                    nc.tensor.matmul(
                        t8c[0:ROPE, 4, :], qpe_s[:, HL, :], identH_sb[:],
                        is_transpose=True, start=False, stop=False)
                    for l in range(1, 4):
                        nc.tensor.matmul(
                            t8c[:, l, :], ckv16[:, l * P:(l + 1) * P],
                            identH_sb[:], is_transpose=True,
                            start=False, stop=(l == 3))
                    nc.scalar.activation(
                        ckvT8h[:, :, :, ts], t8c[:, 0:4, :],
                        mybir.ActivationFunctionType.Copy, scale=CPRE)
                    nc.vector.scalar_tensor_tensor(
                        ckvT8l[:, :, :, ts], t8c[:, 0:4, :], CPRE,
                        ckvT8h[:, :, :, ts],
                        op0=mybir.AluOpType.mult,
                        op1=mybir.AluOpType.subtract)
                    nc.scalar.activation(
                        KP8[0:ROPE, HL, ts], t8c[0:ROPE, 4, :],
                        mybir.ActivationFunctionType.Copy, scale=QKPRE)
                    nc.vector.scalar_tensor_tensor(
                        KP8[0:ROPE, 2 * HL + 1, ts], t8c[0:ROPE, 4, :],
                        QKPRE, KP8[0:ROPE, HL, ts],
                        op0=mybir.AluOpType.mult,
                        op1=mybir.AluOpType.subtract)
                    emit_kabs(t, s1qp, "qp")
                    emit_vabs(t, s1qp, "qp")

                prev = None
                xc = xh = None
                for t in range(NT):
                    if t % 4 == 0:
                        c1 = t // 4
                        xc = s1b.tile([P, NDP, 2, CH], F8, tag="xc",
                                      name="xc")
                        xl = s1b.tile([P, NDP, 2, CH], F8, tag="xl",
                                      name="xl")
                        if c1 == 0:
                            nc.sync.dma_start(
                                out=xc[:], in_=x8[:, :, :, 0:CH])
                            nc.scalar.dma_start(
                                out=xl[:], in_=xlo[:, :, :, 0:CH])
                            nc.sync.dma_start(out=wbk8h_sb[:],
                                              in_=wbk8h[:])
                            nc.sync.dma_start(out=wbk8l_sb[:],
                                              in_=wbk8l[:])
                            nc.scalar.dma_start(out=wbv8h_sb[:],
                                                in_=wbv8h[:])
                            nc.scalar.dma_start(out=wbv8l_sb[:],
                                                in_=wbv8l[:])
                            nc.gpsimd.dma_start(out=maskA_sb[:],
                                                in_=maskA[:])
                        else:
                            for dp in range(NDP):
                                qe = nc.sync if dp % 2 == 0 else nc.scalar
                                qe.dma_start(
                                    out=xc[:, dp, :, :],
                                    in_=x8[:, dp, :, c1 * CH:(c1 + 1) * CH])
                            for dp in range(NDP):
                                qe2 = nc.scalar if dp % 2 == 0 else nc.sync
                                qe2.dma_start(
                                    out=xl[:, dp, :, :],
                                    in_=xlo[:, dp, :, c1 * CH:(c1 + 1) * CH])
                        for ft in range(HL):
                            qp = s1qp.tile([P, CH], FP32, tag="qp",
                                           name="qp")
                            for dp in range(NDP):
                                nc.tensor.matmul(
                                    qp[:],
                                    wq_sb[:, dp, :, ft * P:(ft + 1) * P],
                                    xc[:, dp, :, :],
                                    start=(dp == 0), stop=(dp == NDP - 1),
                                    perf_mode=DRM)
                            cs = slice(c1 * CH, (c1 + 1) * CH)
                            nc.scalar.activation(
                                QP8[:, 0, ft, cs], qp[:],
                                mybir.ActivationFunctionType.Copy,
                                scale=QKPRE / WPRE)
                            nc.vector.scalar_tensor_tensor(
                                QP8[:, 2, ft, cs], qp[:], QKPRE / WPRE,
                                QP8[:, 0, ft, cs],
                                op0=mybir.AluOpType.mult,
                                op1=mybir.AluOpType.subtract)
                    cur = phase_mm(t, xc, xl)
                    if prev is not None:
                        phase_fin(t - 1, *prev)
                    prev = cur
                phase_fin(NT - 1, *prev)

            # ---------------- stage 2: attention -------------------
            with tc.tile_pool(name="s2", bufs=1) as s2, \
                 tc.tile_pool(name="s2b", bufs=3) as s2b, \
                 tc.tile_pool(name="ps_sp", bufs=4, space="PSUM") as ps_sp, \
                 tc.tile_pool(name="ps_op", bufs=3, space="PSUM") as ps_op, \
                 tc.tile_pool(name="ps_sum", bufs=1, space="PSUM") as ps_sum:
                woh_sb = s2.tile([P, 4, DIM], F8, tag="woh", name="woh_sb")
                wol_sb = s2.tile([P, 4, DIM], F8, tag="wol", name="wol_sb")
                nc.sync.dma_start(out=woh_sb[:], in_=wo8h[:])
                nc.sync.dma_start(out=wol_sb[:], in_=wo8l[:])
                def emit_scores(cq, h, ti, bias_t):
                    off = max(ti - 4 * cq, 0) * P
                    sp = ps_sp.tile([P, CH], FP32, tag="sp", name="sp")
                    kc = slice(ti * P, (ti + 1) * P)
                    qs = slice(cq * CH + off, (cq + 1) * CH)
                    # 3 fp8 DR passes: hi.hi, hi-K-residual, Q-residual.hi
                    # KP8 x-slot pairs (K_abs[h], k_pe) have head-dependent
                    # stride; rope rows 64:128 of k_pe slots are zero.
                    k_hi = KP8[:, h:HL + 1:HL - h, kc]
                    k_lo = KP8[:, HL + 1 + h:2 * HL + 2:HL - h, kc]
                    q_hi = QP8[:, 0:2, h, qs]
                    q_lo = QP8[:, 2:4, h, qs]
                    nc.tensor.matmul(sp[:, off:CH], k_hi, q_hi,
                                     start=True, stop=False, perf_mode=DRM)
                    nc.tensor.matmul(sp[:, off:CH], k_lo, q_hi,
                                     start=False, stop=False, perf_mode=DRM)
                    nc.tensor.matmul(sp[:, off:CH], k_hi, q_lo,
                                     start=False, stop=True, perf_mode=DRM)
                    if ti - 4 * cq >= 0:
                        nc.vector.tensor_add(sp[:, off:off + P],
                                             sp[:, off:off + P], maskA_sb[:])
                    pts = s2b.tile([P, CH], FP16, tag="pts",
                                   name="pts", bufs=6)
                    nc.scalar.activation(
                        pts[:, off:CH], sp[:, off:CH],
                        mybir.ActivationFunctionType.Exp,
                        bias=bias_t[:], scale=SCALE / SPROD)
                    return pts, off

                def emit_stage3(cq, d):
                    outp = ps_op.tile([P, CH], FP32, tag="op", name="outp")
                    cols = slice((cq - 1) * CH, cq * CH)
                    chains = [(woh_sb, OTP8h), (woh_sb, OTP8l),
                              (wol_sb, OTP8h)]
                    for ci, (wsb, osb) in enumerate(chains):
                        for j in range(2):
                            nc.tensor.matmul(
                                outp[:],
                                wsb[:, 2 * j:2 * j + 2, d * P:(d + 1) * P],
                                osb[:, 2 * j:2 * j + 2, cols],
                                start=(ci == 0 and j == 0),
                                stop=(ci == 2 and j == 1),
                                perf_mode=DRM)
                    oc = s2b.tile([P, CH], FP16, tag="oc", name="oc",
                                  bufs=3)
                    if d % 2 == 0:
                        nc.vector.tensor_scalar_mul(oc[:], outp[:],
                                                    1.0 / (WPRE * WPRE))
                    else:
                        nc.scalar.activation(
                            oc[:], outp[:],
                            mybir.ActivationFunctionType.Copy,
                            scale=1.0 / (WPRE * WPRE))
                    dq = (nc.sync, nc.gpsimd)[d % 2]
                    dq.dma_start(
                        out=out[d * P:(d + 1) * P, cols],
                        in_=oc[:])

                items = [(cq, h, ti) for cq in range(NCH)
                         for h in range(HL) for ti in range(4 * cq + 4)]
                state = {}
                pend = []
                emit_idx = 0
                defer = {}
                start = 0
                for cq0 in range(NCH):
                    L = HL * (4 * cq0 + 4)
                    if False:
                        for k in range(4):
                            t0 = 4 * (cq0 + 1) + k
                            li = (k + 1) * (L // 4) - 1
                            defer[start + li - 2] = (t0, "k")
                            defer[start + li] = (t0, "v")
                    start += L
                for idx, (cq, h, ti) in enumerate(items):
                    ntk = 4 * cq + 4
                    if ti == 0:
                        state[(cq, h)] = (
                            ps_op.tile([P, CH], FP32, tag="op",
                                       name="op_ps"),
                            ps_sum.tile([P, 4], FP32, tag="sums",
                                        name="sums_t"))
                    op_ps, sums_t = state[(cq, h)]
                    while len(pend) < 4 and emit_idx < len(items):
                        cq2, h2, t2 = items[emit_idx]
                        b_t = nb4 if cq2 == 0 else nb64
                        pend.append(emit_scores(cq2, h2, t2, b_t))
                        emit_idx += 1
                    pts, off = pend.pop(0)
                    if idx in defer:
                        td, kind = defer[idx]
                        if kind == "k":
                            emit_kabs(td, ps_op, "op")
                        else:
                            emit_vabs(td, ps_op, "op")
                    if cq > 0 and ti < 4:
                        emit_stage3(cq, 4 * h + ti)
                    for cc in range(off // P, 4):
                        nc.tensor.matmul(
                            sums_t[:, cc:cc + 1],
                            pts[:, cc * P:(cc + 1) * P],
                            ones16[:],
                            start=(ti == 0 and cc == 0),
                            stop=(ti == ntk - 1 and cc == 3))
                    nc.tensor.matmul(
                        op_ps[:, off:CH],
                        VP[:, ti, h, :],
                        pts[:, off:CH],
                        start=(ti == 0), stop=(ti == ntk - 1))
                    if ti == ntk - 1:
                        # normalization fused into OTP write
                        sums_f = s2b.tile([P, 4], FP32, tag="sums_f",
                                          name="sums_f")
                        nc.vector.tensor_scalar_add(sums_f[:], sums_t[:],
                                                    1e-20)
                        rec_tf = s2b.tile([P, 4], FP32, tag="rec_tf",
                                          name="rec_tf")
                        nc.vector.reciprocal(rec_tf[:], sums_f[:])
                        rec_tp = ps_op.tile([1, 4, P], FP32, tag="op",
                                            name="rec_tp")
                        for cc in range(4):
                            nc.tensor.matmul(rec_tp[:, cc, :],
                                             rec_tf[:, cc:cc + 1],
                                             identF_sb[:],
                                             is_transpose=True,
                                             start=(cc == 0), stop=(cc == 3))
                        rec_r = s2b.tile([1, 4, P], FP16, tag="rec_r",
                                         name="rec_r")
                        nc.scalar.activation(
                            rec_r[:], rec_tp[:],
                            mybir.ActivationFunctionType.Copy, scale=WPRE)
                        recip_bc = s2b.tile([P, CH], FP16, tag="recip_bc",
                                            name="recip_bc")
                        nc.gpsimd.partition_broadcast(recip_bc[:],
                                                      rec_r[0:1, :, :])
                        ot16 = s2b.tile([P, CH], FP16, tag="ot16",
                                        name="ot16", bufs=2)
                        nc.vector.tensor_mul(ot16[:], op_ps[:], recip_bc[:])
                        occ = cq * CH
                        nc.gpsimd.tensor_copy(
                            OTP8h[:, h, occ:occ + CH], ot16[:])
                        nc.vector.scalar_tensor_tensor(
                            OTP8l[:, h, occ:occ + CH], ot16[:], 1.0,
                            OTP8h[:, h, occ:occ + CH],
                            op0=mybir.AluOpType.mult,
                            op1=mybir.AluOpType.subtract)
                        del state[(cq, h)]
                # -------- stage 3 for the final chunk --------
                for d in range(ND):
                    emit_stage3(NCH, d)
    nc.finalize()
    return nc


_NC = None


def _get_nc():
    global _NC
    if _NC is None:
        _NC = build_graph()
    return _NC


def _pair_dim(a):
    """[X*256, N] -> [128, X, 2, N]: pair 128-row subtiles for DoubleRow."""
    x2, n = a.shape
    return np.ascontiguousarray(
        a.reshape(x2 // 256, 2, 128, n).transpose(2, 0, 1, 3))


def _sub_dim(a):
    """[X*128, N] -> [128, X, N]: 128-row subtiles, partition-major."""
    x1, n = a.shape
    return np.ascontiguousarray(a.reshape(x1 // 128, 128, n).transpose(1, 0, 2))


def _prep_core_inputs(x, wq, wkv_a, kv_norm_w, wkv_b, wo, cos, sin):
    """Host-side shard prep. Returns list of 8 in_maps (core = b*4 + g)."""
    perm = np.concatenate([np.arange(0, ROPE, 2), np.arange(1, ROPE, 2)])
    cosf = np.asarray(cos, np.float32)
    sinf = np.asarray(sin, np.float32)
    cosk = np.ascontiguousarray(
        cosf.reshape(NT, P, ROPE // 2).transpose(1, 0, 2).reshape(P, -1))
    sink = np.ascontiguousarray(
        sinf.reshape(NT, P, ROPE // 2).transpose(1, 0, 2).reshape(P, -1))
    cosk5 = np.ascontiguousarray(np.broadcast_to(
        cosk[:, None, :] / WPRE, (P, 5, NT * 32))).astype(np.float16)
    sink5 = np.ascontiguousarray(np.broadcast_to(
        sink[:, None, :] / WPRE, (P, 5, NT * 32))).astype(np.float16)
    identH = np.eye(P, dtype=np.float16)
    k_idx = np.arange(P)[:, None]
    q_idx = np.arange(P)[None, :]
    maskA = np.where(q_idx >= k_idx, 0.0,
                     MASKNEG * SPROD).astype(np.float32)

    w = np.asarray(kv_norm_w, np.float32)
    wq_np = np.asarray(wq, np.float32)
    wkva_np = np.asarray(wkv_a, np.float32)
    wkvb_np = np.asarray(wkv_b, np.float32)
    wo_np = np.asarray(wo, np.float32)
    x_np = np.asarray(x, np.float32)

    def _hilo(rows):
        """[R, DIM] -> paired fp8 hi + residual lo (x WPRE)."""
        wT = np.ascontiguousarray(rows.T) * WPRE
        hi = wT.astype(NPF8)
        lo = (wT - hi.astype(np.float32)).astype(NPF8)
        return _pair_dim(hi.astype(np.float32)).astype(NPF8), \
               _pair_dim(lo.astype(np.float32)).astype(NPF8)

    wkva8h, wkva8l = _hilo(wkva_np[:KV_RANK])
    kpe_rows = wkva_np[KV_RANK:][perm]                    # [64, DIM]

    x8s, xlos = [], []
    for b in range(B):
        xT = np.ascontiguousarray(x_np[b].T)
        hi = xT.astype(NPF8)
        lo = (xT - hi.astype(np.float32)).astype(NPF8)
        x8s.append(_pair_dim(hi.astype(np.float32)).astype(NPF8))
        xlos.append(_pair_dim(lo.astype(np.float32)).astype(NPF8))

    wq_h = wq_np.reshape(H, QK, DIM)
    wb = wkvb_np.reshape(H, NOPE + VDIM, KV_RANK)

    in_maps = []
    _gcache = {}
    for c in range(DP * TP):
        b, g = c // TP, c % TP
        if g in _gcache:
            m = dict(_gcache[g])
            m["x8"] = x8s[b]
            m["xlo"] = xlos[b]
            in_maps.append(m)
            continue
        hs = list(range(g * HL, (g + 1) * HL))
        nope_rows = wq_h[hs, :NOPE].reshape(HL * NOPE, DIM) * WPRE
        wqPa = _pair_dim(np.ascontiguousarray(nope_rows.T)).astype(NPF8)
        rope_rows = wq_h[hs, NOPE:][:, perm].reshape(HL * ROPE, DIM)
        wtok = np.concatenate([rope_rows, kpe_rows], axis=0)   # [320, DIM]
        wtok8h, wtok8l = _hilo(wtok)
        # wbk8[p, lp, j, h, n] = (wb_k[h] w)[n, (2 lp + j)*128 + p] * WBPRE
        def _wb8(rows_slice):
            arr = np.stack([wb[hh, rows_slice] * w[None, :] for hh in hs])
            a = (arr.transpose(2, 0, 1)      # [lat, HL, n]
                 .reshape(2, 2, P, HL, P)    # [lp, j, p, HL, n]
                 .transpose(2, 0, 1, 3, 4))  # [p, lp, j, HL, n]
            a = np.ascontiguousarray(a) * WBPRE
            hi = a.astype(NPF8)
            lo = (a - hi.astype(np.float32)).astype(NPF8)
            return hi, lo
        wbk8ha, wbk8la = _wb8(slice(0, NOPE))
        wbv8ha, wbv8la = _wb8(slice(NOPE, None))
        wo_g = wo_np[:, g * HL * VDIM:(g + 1) * HL * VDIM]
        woT = np.ascontiguousarray(
            wo_g.T.reshape(4, P, DIM).transpose(1, 0, 2)) * WPRE
        wo8ha = woT.astype(NPF8)
        wo8la = (woT - wo8ha.astype(np.float32)).astype(NPF8)
        _gcache[g] = {
            "x8": x8s[b], "xlo": xlos[b], "wqP": wqPa,
            "wtok8h": wtok8h, "wtok8l": wtok8l,
            "wkva8h": wkva8h, "wkva8l": wkva8l,
            "wbk8h": wbk8ha, "wbk8l": wbk8la,
            "wbv8h": wbv8ha, "wbv8l": wbv8la,
            "wo8h": wo8ha, "wo8l": wo8la,
            "cosk5": cosk5, "sink5": sink5,
            "identH": identH, "maskA": maskA,
        }
        in_maps.append(_gcache[g])
    return in_maps


def run(inputs, trace=False, **kw):
    nc = _get_nc()
    in_maps = _prep_core_inputs(**inputs)
    res = run_bass_kernel_spmd(nc, in_maps, list(range(DP * TP)),
                               trace=trace, **kw)
    outs = [r["out"] for r in res.results]
    full = np.empty((B, S, DIM), np.float32)
    for b in range(B):
        acc = outs[b * TP].astype(np.float32)
        for g in range(1, TP):
            acc = acc + outs[b * TP + g].astype(np.float32)
        full[b] = acc.T
    return full, res


def kernel(**inputs):
    inputs = {k: np.asarray(v) for k, v in inputs.items()}
    full, _ = run(inputs)
    return full

